# revision 1
# baseline (speedup 1.0000x reference)
"""Cross-attention Trainium2 kernel (8 NeuronCores, SPMD).

Reference computation (per batch b):
    gate = sigmoid(relu(ctx @ W1 + b1) @ W2 + b2)        # [M, 1]
    ctxg = ctx * gate
    q = x @ Wq; k = ctxg @ Wk; v = ctxg @ Wv             # per head slices of 64
    out = softmax(q k^T / 8) v                           # per head
    y = concat_heads(out) @ Wo + bo                      # [N, 512]

Sharding: 8 cores = 4 batches x 2 query-halves. Each core computes the
FULL output rows for its (batch, 1024-query slice) — no partial sums;
host gather is pure concatenation.

Core-local layout trick: everything is kept transposed (feature dim on
SBUF partitions) so every matmul contraction lands on the partition dim:
    QT[d, i] (d=64/head), KT[d, j], S^T[j, i] = KT_chunk.T @ QT
    E = exp(S^T * scale)  (ScalarE activation doubles as PSUM eviction;
                           no max-subtraction needed: |s| <~ 8 for this data)
    PV: lhsT = [V_h | 1] (ones column) -> out rows 0:64 = V^T E (= O'^T),
        row 64 = colsum(E) = softmax denominator, in the same matmul.
    normalize: O^T = O'^T * (1/denominator) broadcast via a K=1 ones-matmul
        (row 64 becomes exactly 1.0, which then feeds the bias trick below).
    out-proj: y[i, e] = sum_h O^T_h[:, i].T @ Wo_h; head 0 contracts over
        65 rows where row 64 of rhs = bo -> bias added for free.

All matmuls run as float32r (full PE speed at free-dim >= 256, ~fp32
precision).
"""

import os
import sys
from contextlib import ExitStack

import numpy as np

if "/opt/trn_rl_repo" not in sys.path:
    sys.path.insert(0, "/opt/trn_rl_repo")

import concourse.bass as bass
import concourse.mybir as mybir
import concourse.tile as tile
from concourse import bacc
from concourse.bass_utils import run_bass_kernel_spmd
from concourse.masks import make_identity

F32 = mybir.dt.float32
F32R = mybir.dt.float32r
EXPF = mybir.ActivationFunctionType.Exp
RELUF = mybir.ActivationFunctionType.Relu
SIGMF = mybir.ActivationFunctionType.Sigmoid

H = 8          # heads
DH = 64        # dim per head
QD = 512       # query feature dim
CD = 64        # context feature dim
GH = 32        # gate hidden
INNER = H * DH # 512
SCALE = DH ** -0.5


def _r(ap):
    return ap.bitcast(F32R)


def build_core_kernel(nc, NQ=1024, M=2048):
    """Emit the per-core kernel. NQ = queries on this core, M = ctx length."""
    P = 128
    NJC = M // P          # ctx 128-chunks
    NG4 = M // 512        # ctx 512-chunks
    NQC = max(NQ // 512, 1)  # query 512-chunks
    QCW = min(512, NQ)    # query chunk width
    NQ8 = NQ // P         # query 128-chunks
    NKC = QD // P         # 4 qdim 128-chunks

    x_d = nc.dram_tensor("x_in", [NQ, QD], F32, kind="ExternalInput").ap()
    c_d = nc.dram_tensor("ctx_in", [M, CD], F32, kind="ExternalInput").ap()
    wq_d = nc.dram_tensor("wq_in", [QD, INNER], F32, kind="ExternalInput").ap()
    wk_d = nc.dram_tensor("wk_in", [CD, INNER], F32, kind="ExternalInput").ap()
    wv_d = nc.dram_tensor("wv_in", [CD, INNER], F32, kind="ExternalInput").ap()
    wo_d = nc.dram_tensor("wo_in", [INNER, QD], F32, kind="ExternalInput").ap()
    w1_d = nc.dram_tensor("w1_in", [CD, GH], F32, kind="ExternalInput").ap()
    w2_d = nc.dram_tensor("w2_in", [GH, 1], F32, kind="ExternalInput").ap()
    b1_d = nc.dram_tensor("b1_in", [GH, 1], F32, kind="ExternalInput").ap()
    b2_d = nc.dram_tensor("b2_in", [1, 1], F32, kind="ExternalInput").ap()
    bo_d = nc.dram_tensor("bo_in", [1, QD], F32, kind="ExternalInput").ap()
    y_d = nc.dram_tensor("y_out", [NQ, QD], F32, kind="ExternalOutput").ap()

    with TileCtx(nc) as tc, ExitStack() as ctx, \
            nc.allow_low_precision(reason="float32r rounding for PE operands"):
        const = ctx.enter_context(tc.tile_pool(name="const", bufs=1))
        persist = ctx.enter_context(tc.tile_pool(name="persist", bufs=1))
        psum_s = ctx.enter_context(tc.tile_pool(name="psum_s", bufs=2, space="PSUM"))
        psum_pv = ctx.enter_context(tc.tile_pool(name="psum_pv", bufs=2, space="PSUM"))
        psum_pj = ctx.enter_context(tc.tile_pool(name="psum_pj", bufs=2, space="PSUM"))
        early = ExitStack()
        sload = early.enter_context(tc.tile_pool(name="sload", bufs=3))
        gpool = early.enter_context(tc.tile_pool(name="gpool", bufs=2))
        xpool = early.enter_context(tc.tile_pool(name="xpool", bufs=1))

        dma = nc.sync.dma_start

        # ---- constants ----
        ident = const.tile([P, P], F32, tag="ident", name="ident")
        make_identity(nc, ident[:])
        ones_f = const.tile([1, P], F32, tag="ones_f", name="ones_f")
        nc.vector.memset(ones_f[:], 1.0)
        ones = const.tile([1, P], F32R, tag="ones", name="ones")
        nc.vector.tensor_copy(ones[:], ones_f[:])
        onescol_f = const.tile([P, H], F32, tag="onescol_f", name="onescol_f")
        nc.vector.memset(onescol_f[:], 1.0)

        wk_sb = const.tile([CD, INNER], F32R, tag="wk", name="wk")
        dma(wk_sb[:], wk_d[:, :].bitcast(F32R))
        wv_sb = const.tile([CD, INNER], F32R, tag="wv", name="wv")
        dma(wv_sb[:], wv_d[:, :].bitcast(F32R))
        w1_sb = const.tile([CD, GH], F32R, tag="w1", name="w1")
        dma(w1_sb[:], w1_d[:, :].bitcast(F32R))
        w2_sb = const.tile([GH, 1], F32R, tag="w2", name="w2")
        dma(w2_sb[:], w2_d[:, :].bitcast(F32R))
        b1_sb = const.tile([GH, 1], F32, tag="b1", name="b1")
        dma(b1_sb[:], b1_d[:, :])
        b2_sb = const.tile([1, 1], F32, tag="b2", name="b2")
        dma(b2_sb[:], b2_d[:, :])

        # ---- context transpose: ctxT [64, M] ----
        ctxT = persist.tile([CD, M], F32R, tag="ctxT", name="ctxT")
        for g in range(NG4):
            pp = psum_pj.tile([P, 512], F32, tag="pj", name="pj")
            for s in range(4):
                t = g * 4 + s
                cst = sload.tile([P, CD], F32, tag="cld", name="cld")
                dma(cst[:], c_d[t * P:(t + 1) * P, :])
                nc.tensor.transpose(pp[0:CD, s * P:(s + 1) * P], cst[:], ident[:])
            nc.vector.tensor_copy(ctxT[:, g * 512:(g + 1) * 512], pp[0:CD, :])

        # ---- gate + gated context: ctxgT [64, M] ----
        ctxgT = persist.tile([CD, M], F32R, tag="ctxgT", name="ctxgT")
        for g in range(NG4):
            sl = slice(g * 512, (g + 1) * 512)
            pp = psum_pj.tile([P, 512], F32, tag="pj", name="pj")
            nc.tensor.matmul(pp[0:GH, :], _r(w1_sb[:]), _r(ctxT[:, sl]),
                             start=True, stop=True)
            h1 = gpool.tile([GH, 512], F32R, tag="h1", name="h1")
            nc.scalar.activation(h1[:], pp[0:GH, :], RELUF, bias=b1_sb[:])
            pp2 = psum_pj.tile([P, 512], F32, tag="pj", name="pj")
            nc.tensor.matmul(pp2[0:1, :], _r(w2_sb[:]), _r(h1[:]),
                             start=True, stop=True)
            gt = gpool.tile([1, 512], F32R, tag="gt", name="gt")
            nc.scalar.activation(gt[:], pp2[0:1, :], SIGMF, bias=b2_sb[:])
            ppb = psum_pj.tile([P, 512], F32, tag="pj", name="pj")
            nc.tensor.matmul(ppb[0:CD, :], _r(ones[:, 0:CD]), _r(gt[:]),
                             start=True, stop=True)
            nc.vector.tensor_mul(ctxgT[:, sl], ctxT[:, sl], ppb[0:CD, :])

        # ---- K^T, head-pair stacked: KT[pr] [128, M] (rows 0:64 = head 2pr) ----
        KT = [persist.tile([P, M], F32R, tag=f"kt{pr}", name=f"kt{pr}") for pr in range(H // 2)]
        for pr in range(H // 2):
            for g in range(NG4):
                sl = slice(g * 512, (g + 1) * 512)
                pp = psum_pj.tile([P, 512], F32, tag="pj", name="pj")
                nc.tensor.matmul(pp[:], _r(wk_sb[:, pr * P:(pr + 1) * P]),
                                 _r(ctxgT[:, sl]), start=True, stop=True)
                nc.vector.tensor_copy(KT[pr][:, sl], pp[:])

        # ---- V natural, interleaved [V_h | 1] blocks of 65: Vt[t] [128, 520] ----
        Vt = [persist.tile([P, H * (DH + 1)], F32R, tag=f"v{t}", name=f"v{t}") for t in range(NJC)]
        for t in range(NJC):
            vv = Vt[t][:].rearrange("p (h c) -> p h c", c=DH + 1)
            nc.vector.tensor_copy(
                vv[:, :, DH:DH + 1],
                onescol_f[:].rearrange("p (h o) -> p h o", o=1))
            pp = psum_pj.tile([P, 512], F32, tag="pj", name="pj")
            nc.tensor.matmul(pp[:], _r(ctxgT[:, t * P:(t + 1) * P]), _r(wv_sb[:]),
                             start=True, stop=True)
            nc.vector.tensor_copy(
                vv[:, :, 0:DH],
                pp[:].rearrange("p (h c) -> p h c", c=DH))

        # ---- x transpose + Q^T (head-pair stacked): QT[pr] [128, NQ] ----
        # x/wq ride the Activation-engine HWDGE queue so they overlap the
        # ctx-chain DMAs on the SP queue.
        dma2 = nc.scalar.dma_start
        wq_sb = [const.tile([P, INNER], F32R, tag=f"wq{k}", name=f"wq{k}") for k in range(NKC)]
        for k in range(NKC):
            dma2(wq_sb[k][:], wq_d[k * P:(k + 1) * P, :].bitcast(F32R))
        xT = [xpool.tile([P, NQ], F32R, tag=f"xT{k}", name=f"xT{k}") for k in range(NKC)]
        for q8 in range(NQ8):
            xst = sload.tile([P, QD], F32, tag="xld", name="xld")
            dma2(xst[:], x_d[q8 * P:(q8 + 1) * P, :])
            pp = psum_pj.tile([P, 512], F32, tag="pj", name="pj")
            for k in range(NKC):
                nc.tensor.transpose(pp[:, k * P:(k + 1) * P],
                                    xst[:, k * P:(k + 1) * P], ident[:])
            for k in range(NKC):
                nc.vector.tensor_copy(xT[k][:, q8 * P:(q8 + 1) * P],
                                      pp[:, k * P:(k + 1) * P])
        QT = [persist.tile([P, NQ], F32R, tag=f"qt{pr}", name=f"qt{pr}") for pr in range(H // 2)]
        for pr in range(H // 2):
            for qc in range(NQC):
                sl = slice(qc * QCW, (qc + 1) * QCW)
                pp = psum_pj.tile([P, 512], F32, tag="pj", name="pj")
                for k in range(NKC):
                    nc.tensor.matmul(pp[:, 0:QCW],
                                     _r(wq_sb[k][:, pr * P:(pr + 1) * P]),
                                     _r(xT[k][:, sl]),
                                     start=(k == 0), stop=(k == NKC - 1))
                nc.vector.tensor_copy(QT[pr][:, sl], pp[:, 0:QCW])

        early.close()
        epool = ctx.enter_context(tc.tile_pool(name="epool", bufs=3))
        rpool = ctx.enter_context(tc.tile_pool(name="rpool", bufs=2))
        wopool = ctx.enter_context(tc.tile_pool(name="wopool", bufs=1))
        # Wo per head; head 0 gets a 65th row holding bo (bias via ones-row)
        wo_sb = []
        for h in range(H):
            t = wopool.tile([DH + 1 if h == 0 else DH, QD], F32R, tag=f"wo{h}", name=f"wo{h}")
            dma(t[0:DH, :], wo_d[h * DH:(h + 1) * DH, :].bitcast(F32R))
            if h == 0:
                dma(t[DH:DH + 1, :], bo_d[:, :].bitcast(F32R))
            wo_sb.append(t)

        # ---- attention per head ----
        OT = [persist.tile([DH + 1, NQ], F32R, tag=f"ot{h}", name=f"ot{h}") for h in range(H)]
        for h in range(H):
            pr, lo = h // 2, (h % 2) * DH
            pv = [psum_pv.tile([DH + 1, 512], F32, tag="pv", name="pv") for _ in range(NQC)]
            for jc in range(NJC):
                st = psum_s.tile([P, NQC * 512], F32, tag="s", name="st")
                for qc in range(NQC):
                    nc.tensor.matmul(
                        st[:, qc * 512:qc * 512 + QCW],
                        _r(KT[pr][lo:lo + DH, jc * P:(jc + 1) * P]),
                        _r(QT[pr][lo:lo + DH, qc * QCW:(qc + 1) * QCW]),
                        start=True, stop=True)
                et = epool.tile([P, NQC * 512], F32R, tag="e", name="et")
                nc.scalar.activation(et[:], st[:], EXPF, scale=SCALE)
                for qc in range(NQC):
                    nc.tensor.matmul(
                        pv[qc][:, 0:QCW],
                        _r(Vt[jc][:, h * (DH + 1):(h + 1) * (DH + 1)]),
                        _r(et[:, qc * 512:qc * 512 + QCW]),
                        start=(jc == 0), stop=(jc == NJC - 1))
            # fast eviction only — frees the pv banks so the next head's
            # accumulation starts immediately; normalization is deferred.
            for qc in range(NQC):
                sl = slice(qc * QCW, (qc + 1) * QCW)
                nc.vector.tensor_copy(OT[h][:, sl], pv[qc][:, 0:QCW])

        # ---- deferred normalize + output projection, interleaved by qc ----
        # O^T rows 0:64 /= denom (row 64 -> exactly 1.0, feeding the bias
        # trick); then project the q-chunks of this qc while the next qc
        # normalizes.
        for qc in range(NQC):
            sl = slice(qc * QCW, (qc + 1) * QCW)
            for h in range(H):
                rec = rpool.tile([1, 512], F32R, tag="rec", name="rec")
                nc.vector.reciprocal(rec[:, 0:QCW],
                                     OT[h][DH:DH + 1, sl].bitcast(F32))
                rb = psum_pj.tile([DH + 1, 512], F32, tag="pj", name="rb")
                nc.tensor.matmul(rb[:, 0:QCW], _r(ones[:, 0:DH + 1]),
                                 _r(rec[:, 0:QCW]), start=True, stop=True)
                rbs = rpool.tile([DH + 1, 512], F32, tag="rbs", name="rbs")
                nc.vector.tensor_copy(rbs[:, 0:QCW], rb[:, 0:QCW])
                nc.vector.tensor_mul(OT[h][:, sl], OT[h][:, sl].bitcast(F32),
                                     rbs[:, 0:QCW])
            for q8 in range(qc * QCW // P, (qc + 1) * QCW // P):
                po = psum_pj.tile([P, 512], F32, tag="pj", name="pj")
                for h in range(H):
                    kk = DH + 1 if h == 0 else DH
                    nc.tensor.matmul(po[:],
                                     _r(OT[h][0:kk, q8 * P:(q8 + 1) * P]),
                                     _r(wo_sb[h][0:kk, :]),
                                     start=(h == 0), stop=(h == H - 1))
                ost = rpool.tile([P, 512], F32, tag="ost", name="ost")
                nc.vector.tensor_copy(ost[:], po[:])
                dma(y_d[q8 * P:(q8 + 1) * P, :], ost[:])

    return nc


def TileCtx(nc):
    return tile.TileContext(nc)


_NC_CACHE = {}


def _get_compiled(NQ=1024, M=2048):
    key = (NQ, M)
    if key not in _NC_CACHE:
        nc = bacc.Bacc("TRN2", target_bir_lowering=False, debug=False)
        build_core_kernel(nc, NQ=NQ, M=M)
        nc.compile()
        _NC_CACHE[key] = nc
    return _NC_CACHE[key]


def _make_in_maps(inputs):
    x = np.ascontiguousarray(np.asarray(inputs["x"], dtype=np.float32))
    context = np.ascontiguousarray(np.asarray(inputs["context"], dtype=np.float32))
    B, N, _ = x.shape
    NQ = N // 2
    common = {
        "wq_in": np.asarray(inputs["Wq"], np.float32),
        "wk_in": np.asarray(inputs["Wk"], np.float32),
        "wv_in": np.asarray(inputs["Wv"], np.float32),
        "wo_in": np.asarray(inputs["Wo"], np.float32),
        "w1_in": np.asarray(inputs["W1"], np.float32),
        "w2_in": np.asarray(inputs["W2"], np.float32).reshape(GH, 1),
        "b1_in": np.asarray(inputs["b1"], np.float32).reshape(GH, 1),
        "b2_in": np.asarray(inputs["b2"], np.float32).reshape(1, 1),
        "bo_in": np.asarray(inputs["bo"], np.float32).reshape(1, QD),
    }
    in_maps = []
    for c in range(8):
        b, qh = c // 2, c % 2
        m = dict(common)
        m["x_in"] = np.ascontiguousarray(x[b, qh * NQ:(qh + 1) * NQ, :])
        m["ctx_in"] = np.ascontiguousarray(context[b])
        in_maps.append(m)
    return in_maps


def kernel(x, context, Wq, Wk, Wv, W1, b1, W2, b2, Wo, bo):
    x = np.ascontiguousarray(np.asarray(x, dtype=np.float32))
    context = np.ascontiguousarray(np.asarray(context, dtype=np.float32))
    B, N, _ = x.shape
    NQ = N // 2
    M = context.shape[1]
    nc = _get_compiled(NQ=NQ, M=M)
    in_maps = _make_in_maps(dict(
        x=x, context=context, Wq=Wq, Wk=Wk, Wv=Wv, W1=W1, b1=b1, W2=W2,
        b2=b2, Wo=Wo, bo=bo))

    res = run_bass_kernel_spmd(nc, in_maps, list(range(8))).results
    out = np.empty((B, N, QD), dtype=np.float32)
    for c in range(8):
        b, qh = c // 2, c % 2
        out[b, qh * NQ:(qh + 1) * NQ, :] = res[c]["y_out"]
    return out



# revision 15
# speedup vs baseline: 1.8045x; 1.8045x over previous
"""Cross-attention Trainium2 kernel (8 NeuronCores, SPMD) — v2 (bf16).

Reference computation (per batch b):
    gate = sigmoid(relu(ctx @ W1 + b1) @ W2 + b2)        # [M, 1]
    ctxg = ctx * gate
    q = x @ Wq; k = ctxg @ Wk; v = ctxg @ Wv             # per head slices of 64
    out = softmax(q k^T / 8) v                           # per head
    y = concat_heads(out) @ Wo + bo                      # [N, 512]

Sharding: 8 cores = 4 batches x 2 query-halves. Each core computes the
FULL output rows for its (batch, 1024-query slice); host gather is pure
concatenation.

v2 design notes (from baseline trace analysis):
  - All matmul operands are bf16 (1 cyc/row on the PE + fast weight
    load, vs fp32_mode=HIGH observed in the fp32r baseline). PSUM stays
    fp32; rel-err budget (2e-2) is ~10x above bf16 noise.
  - ScalarE exp is the hard floor (~16.8M elements/core, 1 elem/lane/
    cycle @1.2GHz): one N=1024 ACT per (head-pair, jc) covering both
    heads' scores; the sole consumer of ScalarE during attention.
  - S matmuls for a head pair go to disjoint PE row-tiles ((0,0) and
    (64,0), K=64) so the hardware overlaps them.
  - PE program order is software-pipelined (PV lags S by one jc) so the
    strict-FIFO PE queue never stalls behind an ACT it doesn't need.
  - Denominators ride the [V_h | 1] ones-column (row 64 of the PV
    accumulator); all 8 heads' denominators are packed into one [8,512]
    tile so ONE DVE reciprocal serves the whole qc (recip is 8 cyc/elem
    and lane-parallel: [1,512] costs the same as [8,512]).
  - Out-projection contracts head pairs (K=128, full PE) with a K=1
    ones-matmul adding bo.
  - Gate sigmoid is computed as 0.5 + 0.5*tanh(x/2) (tanh lives in the
    same ScalarE table set as exp -> single ACT_TABLE_LOAD).
"""

import sys
from contextlib import ExitStack

import numpy as np

if "/opt/trn_rl_repo" not in sys.path:
    sys.path.insert(0, "/opt/trn_rl_repo")

import concourse.bass as bass
import concourse.mybir as mybir
import concourse.tile as tile
from concourse import bacc
from concourse.bass_utils import run_bass_kernel_spmd
from concourse.masks import make_identity

F32 = mybir.dt.float32
F32R = mybir.dt.float32r
BF16 = mybir.dt.bfloat16
EXPF = mybir.ActivationFunctionType.Exp
RELUF = mybir.ActivationFunctionType.Relu
TANHF = mybir.ActivationFunctionType.Tanh

H = 8          # heads
DH = 64        # dim per head
QD = 512       # query feature dim
CD = 64        # context feature dim
GH = 32        # gate hidden
INNER = H * DH # 512
SCALE = DH ** -0.5
P = 128


def _r(ap):
    return ap.bitcast(F32R)


def build_core_kernel(nc, NQ=1024, M=2048):
    """Emit the per-core kernel. NQ = queries on this core, M = ctx length."""
    NJC = M // P           # ctx 128-chunks (16)
    NG4 = M // 512         # ctx 512-chunks (4)
    NQC = NQ // 512        # query 512-chunks (2)
    NQ8 = NQ // P          # query 128-chunks (8)
    NKC = QD // P          # 4 qdim 128-chunks
    NPR = H // 2           # head pairs (4)

    x_d = nc.dram_tensor("x_in", [NQ, QD], F32, kind="ExternalInput").ap()
    c_d = nc.dram_tensor("ctx_in", [M, CD], F32, kind="ExternalInput").ap()
    wq_d = nc.dram_tensor("wq_in", [QD, INNER], F32, kind="ExternalInput").ap()
    wk_d = nc.dram_tensor("wk_in", [CD, INNER], F32, kind="ExternalInput").ap()
    wv_d = nc.dram_tensor("wv_in", [CD, INNER], F32, kind="ExternalInput").ap()
    wo_d = nc.dram_tensor("wo_in", [INNER, QD], F32, kind="ExternalInput").ap()
    w1_d = nc.dram_tensor("w1_in", [CD, GH], F32, kind="ExternalInput").ap()
    w2_d = nc.dram_tensor("w2_in", [GH, 1], F32, kind="ExternalInput").ap()
    b1_d = nc.dram_tensor("b1_in", [GH, 1], F32, kind="ExternalInput").ap()
    b2_d = nc.dram_tensor("b2_in", [1, 1], F32, kind="ExternalInput").ap()
    bo_d = nc.dram_tensor("bo_in", [1, QD], F32, kind="ExternalInput").ap()
    y_d = nc.dram_tensor("y_out", [NQ, QD], F32, kind="ExternalOutput").ap()

    with tile.TileContext(nc) as tc, ExitStack() as ctx, \
            nc.allow_low_precision(reason="bf16 matmuls; tolerance is 2e-2"):
        const = ctx.enter_context(tc.tile_pool(name="const", bufs=1))
        persist = ctx.enter_context(tc.tile_pool(name="persist", bufs=1))
        psum_s = ctx.enter_context(tc.tile_pool(name="psum_s", bufs=2, space="PSUM"))
        psum_pv = ctx.enter_context(tc.tile_pool(name="psum_pv", bufs=2, space="PSUM"))
        psum_pj = ctx.enter_context(tc.tile_pool(name="psum_pj", bufs=2, space="PSUM"))

        early = ExitStack()
        sload = early.enter_context(tc.tile_pool(name="sload", bufs=3))
        gpool = early.enter_context(tc.tile_pool(name="gpool", bufs=2))
        xpool = early.enter_context(tc.tile_pool(name="xpool", bufs=1))

        dma = nc.sync.dma_start
        dmag = nc.scalar.dma_start

        # ---- constants ----
        ident = const.tile([P, P], F32, tag="ident", name="ident")
        make_identity(nc, ident[:])
        onesrow = const.tile([1, P], BF16, tag="onesrow", name="onesrow")
        nc.vector.memset(onesrow[:], 1.0)
        onescol = const.tile([P, H], BF16, tag="onescol", name="onescol")
        nc.vector.memset(onescol[:], 1.0)

        # indicators for head-pair broadcast, replicated at 32-aligned bases
        # (engine APs need 32-aligned partition bases; matmul lhsT/rhs must
        # share base_partition): row 32pr of indA spreads head 2pr's
        # reciprocal to partitions 0:64, row 32pr of indB spreads head
        # 2pr+1's to partitions 64:128.
        indA = const.tile([P, P], BF16, tag="indA", name="indA")
        nc.vector.memset(indA[:], 0.0)
        indB = const.tile([P, P], BF16, tag="indB", name="indB")
        nc.vector.memset(indB[:], 0.0)
        for pr in range(NPR):
            nc.vector.memset(indA[32 * pr:32 * pr + 1, 0:DH], 1.0)
            nc.vector.memset(indB[32 * pr:32 * pr + 1, DH:P], 1.0)
        halfrow = const.tile([1, CD], BF16, tag="halfrow", name="halfrow")
        nc.vector.memset(halfrow[:], 0.5)
        ones512 = const.tile([1, 512], BF16, tag="ones512", name="ones512")
        nc.vector.memset(ones512[:], 1.0)

        # ---- weight loads (fp32 staging) + bf16 casts ----
        wk_f = const.tile([CD, INNER], F32, tag="wk_f", name="wk_f")
        dma(wk_f[:], wk_d[:, :])
        wk_bf = const.tile([CD, INNER], BF16, tag="wk", name="wk")
        nc.vector.tensor_copy(wk_bf[:], wk_f[:])
        wv_f = const.tile([CD, INNER], F32, tag="wv_f", name="wv_f")
        dma(wv_f[:], wv_d[:, :])
        wv_bf = const.tile([CD, INNER], BF16, tag="wv", name="wv")
        nc.vector.tensor_copy(wv_bf[:], wv_f[:])
        w1_f = const.tile([CD, GH], F32, tag="w1_f", name="w1_f")
        dma(w1_f[:], w1_d[:, :])
        w1_bf = const.tile([CD, GH], BF16, tag="w1", name="w1")
        nc.vector.tensor_copy(w1_bf[:], w1_f[:])
        w2_f = const.tile([GH, 1], F32, tag="w2_f", name="w2_f")
        dma(w2_f[:], w2_d[:, :])
        w2_bf = const.tile([GH, 1], BF16, tag="w2", name="w2")
        nc.vector.tensor_copy(w2_bf[:], w2_f[:])
        b1_sb = const.tile([GH, 1], F32, tag="b1", name="b1")
        dma(b1_sb[:], b1_d[:, :])
        b2_sb = const.tile([1, 1], F32, tag="b2", name="b2")
        dma(b2_sb[:], b2_d[:, :])
        b2h = const.tile([1, 1], F32, tag="b2h", name="b2h")
        nc.scalar.mul(b2h[:], b2_sb[:], 0.5)
        bo_f = const.tile([1, QD], F32, tag="bo_f", name="bo_f")
        dma(bo_f[:], bo_d[:, :])
        bo_bf = const.tile([1, QD], BF16, tag="bo", name="bo")
        nc.vector.tensor_copy(bo_bf[:], bo_f[:])

        wq_bf = [const.tile([P, INNER], BF16, tag=f"wq{k}", name=f"wq{k}")
                 for k in range(NKC)]
        for k in range(NKC):
            wqf = sload.tile([P, INNER], F32, tag="wqf", name="wqf")
            dmag(wqf[:], wq_d[k * P:(k + 1) * P, :])
            nc.vector.tensor_copy(wq_bf[k][:], wqf[:])
        # WoPair[pr]: rows 0:64 = Wo[head 2pr], 64:128 = Wo[head 2pr+1]
        wo_bf = [const.tile([P, QD], BF16, tag=f"wo{pr}", name=f"wo{pr}")
                 for pr in range(NPR)]
        for pr in range(NPR):
            wof = sload.tile([P, QD], F32, tag="wof", name="wof")
            dmag(wof[:], wo_d[pr * P:(pr + 1) * P, :])
            nc.vector.tensor_copy(wo_bf[pr][:], wof[:])

        # ---- context transpose: ctxT [64, M] bf16 ----
        ctxT = persist.tile([CD, M], BF16, tag="ctxT", name="ctxT")
        for g in range(NG4):
            pp = psum_pj.tile([P, 512], F32, tag="pj", name="pj")
            for s in range(4):
                t = g * 4 + s
                cst = sload.tile([P, CD], F32, tag="cld", name="cld")
                dma(cst[:], c_d[t * P:(t + 1) * P, :])
                nc.tensor.transpose(pp[0:CD, s * P:(s + 1) * P],
                                    cst[:], ident[:])
            if g % 2 == 0:
                nc.vector.tensor_copy(ctxT[:, g * 512:(g + 1) * 512], pp[0:CD, :])
            else:
                nc.scalar.copy(ctxT[:, g * 512:(g + 1) * 512], pp[0:CD, :])

        # ---- gate + gated context: ctxgT [64, M] bf16 ----
        # sigmoid(z) computed as 0.5 + 0.5*tanh(z/2): keeps ScalarE in the
        # exp/tanh table set (no sigmoid table load).
        ctxgT = persist.tile([CD, M], BF16, tag="ctxgT", name="ctxgT")
        for g in range(NG4):
            sl = slice(g * 512, (g + 1) * 512)
            pp = psum_pj.tile([P, 512], F32, tag="pj", name="pj")
            nc.tensor.matmul(pp[0:GH, :], w1_bf[:], ctxT[:, sl],
                             start=True, stop=True)
            h1 = gpool.tile([GH, 512], BF16, tag="h1", name="h1")
            nc.scalar.activation(h1[:], pp[0:GH, :], RELUF, bias=b1_sb[:])
            pp2 = psum_pj.tile([P, 512], F32, tag="pj", name="pj")
            nc.tensor.matmul(pp2[0:1, :], w2_bf[:], h1[:],
                             start=True, stop=True)
            gt = gpool.tile([1, 512], BF16, tag="gt", name="gt")
            nc.scalar.activation(gt[:], pp2[0:1, :], TANHF,
                                 bias=b2h[:], scale=0.5)
            # sigmoid = 0.5*tanh + 0.5, built by two accumulated K=1
            # matmuls (broadcast tanh row, then add the constant row)
            ppb = psum_pj.tile([P, 512], F32, tag="pj", name="pj")
            nc.tensor.matmul(ppb[0:CD, :], halfrow[:], gt[:],
                             start=True, stop=False)
            nc.tensor.matmul(ppb[0:CD, :], halfrow[:], ones512[:],
                             start=False, stop=True)
            nc.vector.tensor_mul(ctxgT[:, sl], ctxT[:, sl], ppb[0:CD, :])

        # ---- K^T, head-pair stacked: KT[pr] [128, M] bf16 ----
        KT = [persist.tile([P, M], BF16, tag=f"kt{pr}", name=f"kt{pr}")
              for pr in range(NPR)]
        for pr in range(NPR):
            for g in range(NG4):
                sl = slice(g * 512, (g + 1) * 512)
                pp = psum_pj.tile([P, 512], F32, tag="pj", name="pj")
                nc.tensor.matmul(pp[:], wk_bf[:, pr * P:(pr + 1) * P],
                                 ctxgT[:, sl], start=True, stop=True)
                if g % 2 == 0:
                    nc.vector.tensor_copy(KT[pr][:, sl], pp[:])
                else:
                    nc.scalar.copy(KT[pr][:, sl], pp[:])

        # ---- V natural, interleaved [V_h | 1] blocks of 65: Vt[t] [128, 520] ----
        Vt = [persist.tile([P, H * (DH + 1)], BF16, tag=f"v{t}", name=f"v{t}")
              for t in range(NJC)]
        for t in range(NJC):
            vv = Vt[t][:].rearrange("p (h c) -> p h c", c=DH + 1)
            nc.vector.tensor_copy(
                vv[:, :, DH:DH + 1],
                onescol[:].rearrange("p (h o) -> p h o", o=1))
            pp = psum_pj.tile([P, 512], F32, tag="pj", name="pj")
            nc.tensor.matmul(pp[:], ctxgT[:, t * P:(t + 1) * P], wv_bf[:],
                             start=True, stop=True)
            if t % 2 == 0:
                nc.vector.tensor_copy(vv[:, :, 0:DH],
                                      pp[:].rearrange("p (h c) -> p h c", c=DH))
            else:
                nc.scalar.copy(vv[:, :, 0:DH],
                               pp[:].rearrange("p (h c) -> p h c", c=DH))

        # ---- x transpose + Q^T (head-pair stacked): QT[pr] [128, NQ] bf16 ----
        xT = [xpool.tile([P, NQ], BF16, tag=f"xT{k}", name=f"xT{k}")
              for k in range(NKC)]
        for q8 in range(NQ8):
            xst = sload.tile([P, QD], F32, tag="xld", name="xld")
            dmag(xst[:], x_d[q8 * P:(q8 + 1) * P, :])
            pp = psum_pj.tile([P, 512], F32, tag="pj", name="pj")
            for k in range(NKC):
                nc.tensor.transpose(pp[:, k * P:(k + 1) * P],
                                    xst[:, k * P:(k + 1) * P], ident[:])
            for k in range(NKC):
                if k % 2 == 0:
                    nc.vector.tensor_copy(xT[k][:, q8 * P:(q8 + 1) * P],
                                          pp[:, k * P:(k + 1) * P])
                else:
                    nc.scalar.copy(xT[k][:, q8 * P:(q8 + 1) * P],
                                   pp[:, k * P:(k + 1) * P])
        QT = [persist.tile([P, NQ], BF16, tag=f"qt{pr}", name=f"qt{pr}")
              for pr in range(NPR)]
        for pr in range(NPR):
            for qc in range(NQC):
                sl = slice(qc * 512, (qc + 1) * 512)
                pp = psum_pj.tile([P, 512], F32, tag="pj", name="pj")
                for k in range(NKC):
                    nc.tensor.matmul(pp[:], wq_bf[k][:, pr * P:(pr + 1) * P],
                                     xT[k][:, sl],
                                     start=(k == 0), stop=(k == NKC - 1))
                if pr % 2 == 0:
                    nc.vector.tensor_copy(QT[pr][:, sl], pp[:])
                else:
                    nc.scalar.copy(QT[pr][:, sl], pp[:])

        early.close()
        epool = ctx.enter_context(tc.tile_pool(name="epool", bufs=3))
        opool = ctx.enter_context(tc.tile_pool(name="opool", bufs=4))
        dpool = ctx.enter_context(tc.tile_pool(name="dpool", bufs=2))
        rpool = ctx.enter_context(tc.tile_pool(name="rpool", bufs=2))
        ypool = ctx.enter_context(tc.tile_pool(name="ypool", bufs=2))

        # OTpair[pr] [128, NQ] bf16: normalized O^T for heads (2pr, 2pr+1)
        OTpair = [persist.tile([P, NQ], BF16, tag=f"ot{pr}", name=f"ot{pr}")
                  for pr in range(NPR)]

        for qc in range(NQC):
            qsl = slice(qc * 512, (qc + 1) * 512)
            # packed denominators for this qc at 32-aligned rows: DtA row
            # 32pr = head 2pr, DtB row 32pr = head 2pr+1. Two lane-parallel
            # reciprocals serve all 8 heads. Unused rows preset to 1.0.
            DtA = dpool.tile([P, 512], F32, tag="dtA", name="dtA")
            nc.vector.memset(DtA[:], 1.0)
            DtB = dpool.tile([P, 512], F32, tag="dtB", name="dtB")
            nc.vector.memset(DtB[:], 1.0)
            Oraw = []
            for pr in range(NPR):
                hA, hB = 2 * pr, 2 * pr + 1
                pvA = psum_pv.tile([DH + 1, 512], F32, tag="pv", name="pvA")
                pvB = psum_pv.tile([DH + 1, 512], F32, tag="pv", name="pvB")
                # software pipeline: PV lags S/exp by one jc so the PE
                # FIFO never waits on the ACT it just fed.
                prev_e = None
                for jc in range(NJC):
                    sAB = psum_s.tile([P, 1024], F32, tag="s", name="sAB")
                    nc.tensor.matmul(
                        sAB[:, 0:512],
                        KT[pr][0:DH, jc * P:(jc + 1) * P],
                        QT[pr][0:DH, qsl], start=True, stop=True)
                    nc.tensor.matmul(
                        sAB[:, 512:1024],
                        KT[pr][DH:P, jc * P:(jc + 1) * P],
                        QT[pr][DH:P, qsl], start=True, stop=True)
                    eAB = epool.tile([P, 1024], BF16, tag="e", name="eAB")
                    nc.scalar.activation(eAB[:], sAB[:], EXPF, scale=SCALE)
                    if prev_e is not None:
                        pj, pe = prev_e
                        nc.tensor.matmul(
                            pvA[:], Vt[pj][:, hA * (DH + 1):(hA + 1) * (DH + 1)],
                            pe[:, 0:512], start=(pj == 0), stop=False)
                        nc.tensor.matmul(
                            pvB[:], Vt[pj][:, hB * (DH + 1):(hB + 1) * (DH + 1)],
                            pe[:, 512:1024], start=(pj == 0), stop=False)
                    prev_e = (jc, eAB)
                pj, pe = prev_e
                nc.tensor.matmul(
                    pvA[:], Vt[pj][:, hA * (DH + 1):(hA + 1) * (DH + 1)],
                    pe[:, 0:512], start=False, stop=True)
                nc.tensor.matmul(
                    pvB[:], Vt[pj][:, hB * (DH + 1):(hB + 1) * (DH + 1)],
                    pe[:, 512:1024], start=False, stop=True)
                # evict: O' rows to SBUF fp32; denominators to packed tile
                orw = opool.tile([P, 512], F32, tag="oraw", name="oraw")
                nc.vector.tensor_copy(orw[0:DH, :], pvA[0:DH, :])
                nc.vector.tensor_copy(orw[DH:P, :], pvB[0:DH, :])
                nc.vector.tensor_copy(DtA[32 * pr:32 * pr + 1, :], pvA[DH:DH + 1, :])
                nc.vector.tensor_copy(DtB[32 * pr:32 * pr + 1, :], pvB[DH:DH + 1, :])
                Oraw.append(orw)
            # two reciprocals for all 8 heads (lane-parallel)
            RA = rpool.tile([P, 512], BF16, tag="recA", name="recA")
            nc.vector.reciprocal(RA[:], DtA[:])
            RB = rpool.tile([P, 512], BF16, tag="recB", name="recB")
            nc.vector.reciprocal(RB[:], DtB[:])
            for pr in range(NPR):
                bc = psum_pj.tile([P, 512], F32, tag="pj", name="bc")
                nc.tensor.matmul(bc[:], indA[32 * pr:32 * pr + 1, :],
                                 RA[32 * pr:32 * pr + 1, :],
                                 start=True, stop=False,
                                 tile_position=(32 * pr, 0))
                nc.tensor.matmul(bc[:], indB[32 * pr:32 * pr + 1, :],
                                 RB[32 * pr:32 * pr + 1, :],
                                 start=False, stop=True,
                                 tile_position=(32 * pr, 0))
                nc.vector.tensor_mul(OTpair[pr][:, qsl], Oraw[pr][:], bc[:])
            # out-projection for this qc: per 128-query chunk, contract
            # head pairs (K=128) + K=1 ones-matmul for the bias.
            for q8 in range(qc * (512 // P), (qc + 1) * (512 // P)):
                po = psum_pj.tile([P, 512], F32, tag="pj", name="po")
                nc.tensor.matmul(po[:], onesrow[:], bo_bf[:],
                                 start=True, stop=False)
                for pr in range(NPR):
                    nc.tensor.matmul(po[:],
                                     OTpair[pr][:, q8 * P:(q8 + 1) * P],
                                     wo_bf[pr][:],
                                     start=False, stop=(pr == NPR - 1))
                ost = ypool.tile([P, 512], F32, tag="ost", name="ost")
                nc.vector.tensor_copy(ost[:], po[:])
                dma(y_d[q8 * P:(q8 + 1) * P, :], ost[:])

    return nc


_NC_CACHE = {}


def _get_compiled(NQ=1024, M=2048):
    key = (NQ, M)
    if key not in _NC_CACHE:
        nc = bacc.Bacc("TRN2", target_bir_lowering=False, debug=False)
        build_core_kernel(nc, NQ=NQ, M=M)
        nc.compile()
        _NC_CACHE[key] = nc
    return _NC_CACHE[key]


def _make_in_maps(inputs):
    x = np.ascontiguousarray(np.asarray(inputs["x"], dtype=np.float32))
    context = np.ascontiguousarray(np.asarray(inputs["context"], dtype=np.float32))
    B, N, _ = x.shape
    NQ = N // 2
    common = {
        "wq_in": np.asarray(inputs["Wq"], np.float32),
        "wk_in": np.asarray(inputs["Wk"], np.float32),
        "wv_in": np.asarray(inputs["Wv"], np.float32),
        "wo_in": np.asarray(inputs["Wo"], np.float32),
        "w1_in": np.asarray(inputs["W1"], np.float32),
        "w2_in": np.asarray(inputs["W2"], np.float32).reshape(GH, 1),
        "b1_in": np.asarray(inputs["b1"], np.float32).reshape(GH, 1),
        "b2_in": np.asarray(inputs["b2"], np.float32).reshape(1, 1),
        "bo_in": np.asarray(inputs["bo"], np.float32).reshape(1, QD),
    }
    in_maps = []
    for c in range(8):
        b, qh = c // 2, c % 2
        m = dict(common)
        m["x_in"] = np.ascontiguousarray(x[b, qh * NQ:(qh + 1) * NQ, :])
        m["ctx_in"] = np.ascontiguousarray(context[b])
        in_maps.append(m)
    return in_maps


def kernel(x, context, Wq, Wk, Wv, W1, b1, W2, b2, Wo, bo):
    x = np.ascontiguousarray(np.asarray(x, dtype=np.float32))
    context = np.ascontiguousarray(np.asarray(context, dtype=np.float32))
    B, N, _ = x.shape
    NQ = N // 2
    M = context.shape[1]
    nc = _get_compiled(NQ=NQ, M=M)
    in_maps = _make_in_maps(dict(
        x=x, context=context, Wq=Wq, Wk=Wk, Wv=Wv, W1=W1, b1=b1, W2=W2,
        b2=b2, Wo=Wo, bo=bo))

    res = run_bass_kernel_spmd(nc, in_maps, list(range(8))).results
    out = np.empty((B, N, QD), dtype=np.float32)
    for c in range(8):
        b, qh = c // 2, c % 2
        out[b, qh * NQ:(qh + 1) * NQ, :] = res[c]["y_out"]
    return out


# revision 17
# speedup vs baseline: 1.8370x; 1.0180x over previous
"""Cross-attention Trainium2 kernel (8 NeuronCores, SPMD) — v3 (bf16).

Reference computation (per batch b):
    gate = sigmoid(relu(ctx @ W1 + b1) @ W2 + b2)        # [M, 1]
    ctxg = ctx * gate
    q = x @ Wq; k = ctxg @ Wk; v = ctxg @ Wv             # per head slices of 64
    out = softmax(q k^T / 8) v                           # per head
    y = concat_heads(out) @ Wo + bo                      # [N, 512]

Sharding: 8 cores = 4 batches x 2 query-halves. Each core computes the
FULL output rows for its (batch, 1024-query slice); host gather is pure
concatenation.

Design (from trace analysis of the fp32r baseline and v2):
  - All matmul operands bf16 (1 cyc/row on the PE); PSUM stays fp32.
  - ScalarE exp is the hard floor (~16.8M elements/core @ 1 elem/lane/
    cycle): one N=1024 ACT per (head-pair, jc); nothing else runs on
    ScalarE during attention; PE program order is software-pipelined
    (PV lags S by one jc) so the strict-FIFO PE queue never waits on
    the ACT it just fed.
  - S matmuls for a head pair land on disjoint PE row-tiles ((0,0) /
    (64,0), K=64) and run concurrently.
  - x^T / ctx^T come from the DMA xbar transpose: fp32 DRAM viewed as
    uint16 and transposed puts the high halves — i.e. truncated bf16 —
    on odd partitions; an SBUF-to-SBUF DMA extracts them. Zero
    PE/DVE/ScalarE cost for the transposes.
  - A dummy-matmul warmup burst at t=0 (during the input DMAs) lifts
    the PE HAM clock gate from 1.2 to 2.4 GHz before real work starts.
  - Denominators ride the [V_h | 1] ones-column; all 16 (head, half)
    denominators pack into one [128,1024] tile at 32-aligned rows ->
    ONE lane-parallel reciprocal_approx_fast per qc.
  - qc0's normalization + out-projection are emitted inside qc1's
    attention stream, filling the PE slack under the ScalarE-paced
    steady state instead of stalling it at the boundary.
"""

import sys
from contextlib import ExitStack

import numpy as np

if "/opt/trn_rl_repo" not in sys.path:
    sys.path.insert(0, "/opt/trn_rl_repo")

import concourse.bass as bass
import concourse.mybir as mybir
import concourse.tile as tile
from concourse import bacc
from concourse.bass_utils import run_bass_kernel_spmd

F32 = mybir.dt.float32
F32R = mybir.dt.float32r
BF16 = mybir.dt.bfloat16
U16 = mybir.dt.uint16
EXPF = mybir.ActivationFunctionType.Exp
RELUF = mybir.ActivationFunctionType.Relu
TANHF = mybir.ActivationFunctionType.Tanh

H = 8          # heads
DH = 64        # dim per head
QD = 512       # query feature dim
CD = 64        # context feature dim
GH = 32        # gate hidden
INNER = H * DH # 512
SCALE = DH ** -0.5
P = 128


def build_core_kernel(nc, NQ=1024, M=2048):
    """Emit the per-core kernel. NQ = queries on this core, M = ctx length."""
    NJC = M // P           # ctx 128-chunks (16)
    NQC = NQ // 512        # query 512-chunks (2)
    NKC = QD // P          # 4 qdim 128-chunks
    NPR = H // 2           # head pairs (4)
    NXU = QD // DH         # 8 u16-transpose chunks of x

    x_d = nc.dram_tensor("x_in", [NQ, QD], F32, kind="ExternalInput").ap()
    c_d = nc.dram_tensor("ctx_in", [M, CD], F32, kind="ExternalInput").ap()
    wq_d = nc.dram_tensor("wq_in", [QD, INNER], F32, kind="ExternalInput").ap()
    wk_d = nc.dram_tensor("wk_in", [CD, INNER], F32, kind="ExternalInput").ap()
    wv_d = nc.dram_tensor("wv_in", [CD, INNER], F32, kind="ExternalInput").ap()
    wo_d = nc.dram_tensor("wo_in", [INNER, QD], F32, kind="ExternalInput").ap()
    w1_d = nc.dram_tensor("w1_in", [CD, GH], F32, kind="ExternalInput").ap()
    w2_d = nc.dram_tensor("w2_in", [GH, 1], F32, kind="ExternalInput").ap()
    b1_d = nc.dram_tensor("b1_in", [GH, 1], F32, kind="ExternalInput").ap()
    b2_d = nc.dram_tensor("b2_in", [1, 1], F32, kind="ExternalInput").ap()
    bo_d = nc.dram_tensor("bo_in", [1, QD], F32, kind="ExternalInput").ap()
    y_d = nc.dram_tensor("y_out", [NQ, QD], F32, kind="ExternalOutput").ap()

    with tile.TileContext(nc) as tc, ExitStack() as ctx, \
            nc.allow_low_precision(reason="bf16 matmuls; tolerance is 2e-2"):
        const = ctx.enter_context(tc.tile_pool(name="const", bufs=1))
        persist = ctx.enter_context(tc.tile_pool(name="persist", bufs=1))
        psum_s = ctx.enter_context(tc.tile_pool(name="psum_s", bufs=2, space="PSUM"))
        psum_pv = ctx.enter_context(tc.tile_pool(name="psum_pv", bufs=2, space="PSUM"))
        psum_pj = ctx.enter_context(tc.tile_pool(name="psum_pj", bufs=2, space="PSUM"))

        early = ExitStack()
        sload = early.enter_context(tc.tile_pool(name="sload", bufs=3))
        gpool = early.enter_context(tc.tile_pool(name="gpool", bufs=2))
        xpool = early.enter_context(tc.tile_pool(name="xpool", bufs=1))

        dma = nc.sync.dma_start
        dmag = nc.scalar.dma_start

        # ---- constants (DMA-independent) ----
        onesrow = const.tile([1, P], BF16, tag="onesrow", name="onesrow")
        nc.vector.memset(onesrow[:], 1.0)
        onescol = const.tile([P, H], BF16, tag="onescol", name="onescol")
        nc.vector.memset(onescol[:], 1.0)
        halfrow = const.tile([1, CD], BF16, tag="halfrow", name="halfrow")
        nc.vector.memset(halfrow[:], 0.5)
        ones512 = const.tile([1, 512], BF16, tag="ones512", name="ones512")
        nc.vector.memset(ones512[:], 1.0)
        indA = const.tile([P, P], BF16, tag="indA", name="indA")
        nc.vector.memset(indA[:], 0.0)
        indB = const.tile([P, P], BF16, tag="indB", name="indB")
        nc.vector.memset(indB[:], 0.0)
        for pr in range(NPR):
            nc.vector.memset(indA[32 * pr:32 * pr + 1, 0:DH], 1.0)
            nc.vector.memset(indB[32 * pr:32 * pr + 1, DH:P], 1.0)
        warm_sb = const.tile([P, 512], BF16, tag="warm", name="warm")
        nc.vector.memset(warm_sb[:], 0.5)

        # ---- PE warmup: dummy matmuls while the input DMAs stream in.
        # HAM un-throttles (1.2 -> 2.4 GHz) after ~3.4us of sustained PE
        # activity; these keep the array busy until real work arrives.
        wp = psum_s.tile([P, 1024], F32, tag="s", name="warmps")
        for w in range(14):
            nc.tensor.matmul(wp[0:H, 0:512], onescol[:], warm_sb[:],
                             start=True, stop=True)

        # ---- input transposes via DMA xbar (u16 view of fp32; odd
        # partitions carry the high halves = truncated bf16) ----
        ctmp = xpool.tile([P, M], U16, tag="ctmp", name="ctmp")
        dma(ctmp[:], c_d.bitcast(U16))
        ctxT = persist.tile([CD, M], BF16, tag="ctxT", name="ctxT")
        dma(ctxT[:].bitcast(U16),
            ctmp[:].rearrange("(a b) f -> a b f", b=2)[:, 1, :])

        xtmp = [xpool.tile([P, NQ], U16, tag=f"xtmp{c}", name=f"xtmp{c}")
                for c in range(NXU)]
        for c in range(NXU):
            dmag(xtmp[c][:], x_d.bitcast(U16)[:, c * P:(c + 1) * P])
        xT = [xpool.tile([P, NQ], BF16, tag=f"xT{k}", name=f"xT{k}")
              for k in range(NKC)]
        for k in range(NKC):
            dma(xT[k][0:DH, :].bitcast(U16),
                xtmp[2 * k][:].rearrange("(a b) f -> a b f", b=2)[:, 1, :])
            dma(xT[k][DH:P, :].bitcast(U16),
                xtmp[2 * k + 1][:].rearrange("(a b) f -> a b f", b=2)[:, 1, :])

        # ---- weight loads (fp32 staging) + bf16 casts ----
        w1_f = const.tile([CD, GH], F32, tag="w1_f", name="w1_f")
        dma(w1_f[:], w1_d[:, :])
        w1_bf = const.tile([CD, GH], BF16, tag="w1", name="w1")
        nc.vector.tensor_copy(w1_bf[:], w1_f[:])
        w2_f = const.tile([GH, 1], F32, tag="w2_f", name="w2_f")
        dma(w2_f[:], w2_d[:, :])
        w2_bf = const.tile([GH, 1], BF16, tag="w2", name="w2")
        nc.vector.tensor_copy(w2_bf[:], w2_f[:])
        b1_sb = const.tile([GH, 1], F32, tag="b1", name="b1")
        dma(b1_sb[:], b1_d[:, :])
        b2_sb = const.tile([1, 1], F32, tag="b2", name="b2")
        dma(b2_sb[:], b2_d[:, :])
        b2h = const.tile([1, 1], F32, tag="b2h", name="b2h")
        nc.scalar.mul(b2h[:], b2_sb[:], 0.5)
        wk_f = const.tile([CD, INNER], F32, tag="wk_f", name="wk_f")
        dma(wk_f[:], wk_d[:, :])
        wk_bf = const.tile([CD, INNER], BF16, tag="wk", name="wk")
        nc.vector.tensor_copy(wk_bf[:], wk_f[:])
        wv_f = const.tile([CD, INNER], F32, tag="wv_f", name="wv_f")
        dma(wv_f[:], wv_d[:, :])
        wv_bf = const.tile([CD, INNER], BF16, tag="wv", name="wv")
        nc.vector.tensor_copy(wv_bf[:], wv_f[:])
        bo_f = const.tile([1, QD], F32, tag="bo_f", name="bo_f")
        dma(bo_f[:], bo_d[:, :])
        bo_bf = const.tile([1, QD], BF16, tag="bo", name="bo")
        nc.vector.tensor_copy(bo_bf[:], bo_f[:])

        wq_bf = [const.tile([P, INNER], BF16, tag=f"wq{k}", name=f"wq{k}")
                 for k in range(NKC)]
        for k in range(NKC):
            wqf = sload.tile([P, INNER], F32, tag="wqf", name="wqf")
            dmag(wqf[:], wq_d[k * P:(k + 1) * P, :])
            nc.vector.tensor_copy(wq_bf[k][:], wqf[:])
        # WoPair[pr]: rows 0:64 = Wo[head 2pr], 64:128 = Wo[head 2pr+1]
        wo_bf = [const.tile([P, QD], BF16, tag=f"wo{pr}", name=f"wo{pr}")
                 for pr in range(NPR)]
        for pr in range(NPR):
            wof = sload.tile([P, QD], F32, tag="wof", name="wof")
            dmag(wof[:], wo_d[pr * P:(pr + 1) * P, :])
            nc.vector.tensor_copy(wo_bf[pr][:], wof[:])

        # ---- gate + gated context: ctxgT [64, M] bf16 ----
        # sigmoid(z) = 0.5 + 0.5*tanh(z/2): stays in the exp/tanh ScalarE
        # table set (no sigmoid table load).
        ctxgT = persist.tile([CD, M], BF16, tag="ctxgT", name="ctxgT")
        for g in range(M // 512):
            sl = slice(g * 512, (g + 1) * 512)
            pool = psum_pj if g % 2 == 0 else psum_s
            pp = pool.tile([P, 512] if g % 2 == 0 else [P, 1024], F32,
                           tag="pj" if g % 2 == 0 else "s", name="gpp")
            nc.tensor.matmul(pp[0:GH, 0:512], w1_bf[:], ctxT[:, sl],
                             start=True, stop=True)
            h1 = gpool.tile([GH, 512], BF16, tag="h1", name="h1")
            nc.scalar.activation(h1[:], pp[0:GH, 0:512], RELUF, bias=b1_sb[:])
            pp2 = pool.tile([P, 512] if g % 2 == 0 else [P, 1024], F32,
                            tag="pj" if g % 2 == 0 else "s", name="gpp2")
            nc.tensor.matmul(pp2[0:1, 0:512], w2_bf[:], h1[:],
                             start=True, stop=True)
            gt = gpool.tile([1, 512], BF16, tag="gt", name="gt")
            nc.scalar.activation(gt[:], pp2[0:1, 0:512], TANHF,
                                 bias=b2h[:], scale=0.5)
            ppb = pool.tile([P, 512] if g % 2 == 0 else [P, 1024], F32,
                            tag="pj" if g % 2 == 0 else "s", name="gppb")
            nc.tensor.matmul(ppb[0:CD, 0:512], halfrow[:], gt[:],
                             start=True, stop=False)
            nc.tensor.matmul(ppb[0:CD, 0:512], halfrow[:], ones512[:],
                             start=False, stop=True)
            nc.vector.tensor_mul(ctxgT[:, sl], ctxT[:, sl], ppb[0:CD, 0:512])

        # ---- K^T, head-pair stacked: KT[pr] [128, M] bf16 ----
        KT = [persist.tile([P, M], BF16, tag=f"kt{pr}", name=f"kt{pr}")
              for pr in range(NPR)]
        for pr in range(NPR):
            for gg in range(M // 1024):
                sl2 = slice(gg * 1024, (gg + 1) * 1024)
                pp = psum_s.tile([P, 1024], F32, tag="s", name="ktp")
                for half in range(2):
                    sl = slice(gg * 1024 + half * 512, gg * 1024 + half * 512 + 512)
                    nc.tensor.matmul(pp[:, half * 512:half * 512 + 512],
                                     wk_bf[:, pr * P:(pr + 1) * P],
                                     ctxgT[:, sl], start=True, stop=True)
                if (pr + gg) % 2 == 0:
                    nc.vector.tensor_copy(KT[pr][:, sl2], pp[:])
                else:
                    nc.scalar.copy(KT[pr][:, sl2], pp[:])

        # ---- V natural, interleaved [V_h | 1] blocks of 65: Vt[t] [128, 520] ----
        Vt = [persist.tile([P, H * (DH + 1)], BF16, tag=f"v{t}", name=f"v{t}")
              for t in range(NJC)]
        for t in range(NJC):
            if t % 2 == 0:
                pp = psum_pj.tile([P, 512], F32, tag="pj", name="vtp")
                src = pp[:]
            else:
                pp = psum_s.tile([P, 1024], F32, tag="s", name="vtp")
                src = pp[:, 0:512]
            nc.tensor.matmul(src, ctxgT[:, t * P:(t + 1) * P], wv_bf[:],
                             start=True, stop=True)
            vv = Vt[t][:].rearrange("p (h c) -> p h c", c=DH + 1)
            nc.vector.tensor_copy(
                vv[:, :, DH:DH + 1],
                onescol[:].rearrange("p (h o) -> p h o", o=1))
            if t % 2 == 0:
                nc.vector.tensor_copy(
                    vv[:, :, 0:DH], src.rearrange("p (h c) -> p h c", c=DH))
            else:
                nc.scalar.copy(
                    vv[:, :, 0:DH], src.rearrange("p (h c) -> p h c", c=DH))

        # ---- Q^T (head-pair stacked): QT[pr] [128, NQ] bf16 ----
        QT = [persist.tile([P, NQ], BF16, tag=f"qt{pr}", name=f"qt{pr}")
              for pr in range(NPR)]
        for pr in range(NPR):
            pp = psum_s.tile([P, 1024], F32, tag="s", name="qtp")
            for qc in range(NQC):
                for k in range(NKC):
                    nc.tensor.matmul(pp[:, qc * 512:qc * 512 + 512],
                                     wq_bf[k][:, pr * P:(pr + 1) * P],
                                     xT[k][:, qc * 512:qc * 512 + 512],
                                     start=(k == 0), stop=(k == NKC - 1))
            if pr % 2 == 0:
                nc.vector.tensor_copy(QT[pr][:], pp[:])
            else:
                nc.scalar.copy(QT[pr][:], pp[:])

        early.close()
        epool = ctx.enter_context(tc.tile_pool(name="epool", bufs=3))
        opool = ctx.enter_context(tc.tile_pool(name="opool", bufs=8))
        dpool = ctx.enter_context(tc.tile_pool(name="dpool", bufs=2))
        rpool = ctx.enter_context(tc.tile_pool(name="rpool", bufs=2))
        bpool = ctx.enter_context(tc.tile_pool(name="bpool", bufs=2))
        ypool = ctx.enter_context(tc.tile_pool(name="ypool", bufs=2))

        # OTpair[pr] [128, NQ] bf16: normalized O^T for heads (2pr, 2pr+1)
        OTpair = [persist.tile([P, NQ], BF16, tag=f"ot{pr}", name=f"ot{pr}")
                  for pr in range(NPR)]
        # per-qc state
        Dt = [None] * NQC
        Oraw = [[None] * NPR for _ in range(NQC)]

        def attention_block(qc, pr):
            qsl = slice(qc * 512, (qc + 1) * 512)
            hA, hB = 2 * pr, 2 * pr + 1
            if pr == 0:
                Dt[qc] = dpool.tile([P, 1024], F32, tag="dt", name="dt")
                nc.vector.memset(Dt[qc][:], 1.0)
            pvA = psum_pv.tile([DH + 1, 512], F32, tag="pv", name="pvA")
            pvB = psum_pv.tile([DH + 1, 512], F32, tag="pv", name="pvB")
            # software pipeline: PV lags S/exp by one jc so the PE FIFO
            # never waits on the ACT it just fed.
            prev_e = None
            for jc in range(NJC):
                sAB = psum_s.tile([P, 1024], F32, tag="s", name="sAB")
                nc.tensor.matmul(
                    sAB[:, 0:512],
                    KT[pr][0:DH, jc * P:(jc + 1) * P],
                    QT[pr][0:DH, qsl], start=True, stop=True)
                nc.tensor.matmul(
                    sAB[:, 512:1024],
                    KT[pr][DH:P, jc * P:(jc + 1) * P],
                    QT[pr][DH:P, qsl], start=True, stop=True)
                eAB = epool.tile([P, 1024], BF16, tag="e", name="eAB")
                nc.scalar.activation(eAB[:], sAB[:], EXPF, scale=SCALE)
                if prev_e is not None:
                    pj, pe = prev_e
                    nc.tensor.matmul(
                        pvA[:], Vt[pj][:, hA * (DH + 1):(hA + 1) * (DH + 1)],
                        pe[:, 0:512], start=(pj == 0), stop=False)
                    nc.tensor.matmul(
                        pvB[:], Vt[pj][:, hB * (DH + 1):(hB + 1) * (DH + 1)],
                        pe[:, 512:1024], start=(pj == 0), stop=False)
                prev_e = (jc, eAB)
            pj, pe = prev_e
            nc.tensor.matmul(
                pvA[:], Vt[pj][:, hA * (DH + 1):(hA + 1) * (DH + 1)],
                pe[:, 0:512], start=False, stop=True)
            nc.tensor.matmul(
                pvB[:], Vt[pj][:, hB * (DH + 1):(hB + 1) * (DH + 1)],
                pe[:, 512:1024], start=False, stop=True)
            # evict O' rows to SBUF fp32; denominators into the packed
            # tile (A heads in cols 0:512, B heads in 512:1024, at
            # 32-aligned rows).
            orw = opool.tile([P, 512], F32, tag="oraw", name="oraw")
            nc.vector.tensor_copy(orw[0:DH, :], pvA[0:DH, :])
            nc.vector.tensor_copy(orw[DH:P, :], pvB[0:DH, :])
            nc.vector.tensor_copy(Dt[qc][32 * pr:32 * pr + 1, 0:512],
                                  pvA[DH:DH + 1, :])
            nc.vector.tensor_copy(Dt[qc][32 * pr:32 * pr + 1, 512:1024],
                                  pvB[DH:DH + 1, :])
            Oraw[qc][pr] = orw

        def norm_and_project(qc):
            qsl = slice(qc * 512, (qc + 1) * 512)
            # one lane-parallel approx reciprocal for all 16 denominators
            Rf = rpool.tile([P, 1024], F32, tag="rf", name="rf")
            nc.vector.reciprocal_approx_fast(Rf[:], Dt[qc][:])
            Rb = rpool.tile([P, 1024], BF16, tag="rb", name="rb")
            nc.vector.tensor_copy(Rb[:], Rf[:])
            for pr in range(NPR):
                bc = psum_pj.tile([P, 512], F32, tag="pj", name="bc")
                nc.tensor.matmul(bc[:], indA[32 * pr:32 * pr + 1, :],
                                 Rb[32 * pr:32 * pr + 1, 0:512],
                                 start=True, stop=False,
                                 tile_position=(32 * pr, 0))
                nc.tensor.matmul(bc[:], indB[32 * pr:32 * pr + 1, :],
                                 Rb[32 * pr:32 * pr + 1, 512:1024],
                                 start=False, stop=True,
                                 tile_position=(32 * pr, 0))
                nc.vector.tensor_mul(OTpair[pr][:, qsl], Oraw[qc][pr][:], bc[:])
            # out-projection: per 128-query chunk, contract head pairs
            # (K=128) after a K=1 ones-matmul seeds the bias.
            for q8 in range(qc * (512 // P), (qc + 1) * (512 // P)):
                po = psum_pj.tile([P, 512], F32, tag="pj", name="po")
                nc.tensor.matmul(po[:], onesrow[:], bo_bf[:],
                                 start=True, stop=False)
                for pr in range(NPR):
                    nc.tensor.matmul(po[:],
                                     OTpair[pr][:, q8 * P:(q8 + 1) * P],
                                     wo_bf[pr][:],
                                     start=False, stop=(pr == NPR - 1))
                ost = ypool.tile([P, 512], F32, tag="ost", name="ost")
                nc.vector.tensor_copy(ost[:], po[:])
                dma(y_d[q8 * P:(q8 + 1) * P, :], ost[:])

        for pr in range(NPR):
            attention_block(0, pr)
        attention_block(1, 0)
        norm_and_project(0)   # emitted inside qc1's attention stream
        for pr in range(1, NPR):
            attention_block(1, pr)
        norm_and_project(1)

    return nc


_NC_CACHE = {}


def _get_compiled(NQ=1024, M=2048):
    key = (NQ, M)
    if key not in _NC_CACHE:
        nc = bacc.Bacc("TRN2", target_bir_lowering=False, debug=False)
        build_core_kernel(nc, NQ=NQ, M=M)
        nc.compile()
        _NC_CACHE[key] = nc
    return _NC_CACHE[key]


def _make_in_maps(inputs):
    x = np.ascontiguousarray(np.asarray(inputs["x"], dtype=np.float32))
    context = np.ascontiguousarray(np.asarray(inputs["context"], dtype=np.float32))
    B, N, _ = x.shape
    NQ = N // 2
    common = {
        "wq_in": np.asarray(inputs["Wq"], np.float32),
        "wk_in": np.asarray(inputs["Wk"], np.float32),
        "wv_in": np.asarray(inputs["Wv"], np.float32),
        "wo_in": np.asarray(inputs["Wo"], np.float32),
        "w1_in": np.asarray(inputs["W1"], np.float32),
        "w2_in": np.asarray(inputs["W2"], np.float32).reshape(GH, 1),
        "b1_in": np.asarray(inputs["b1"], np.float32).reshape(GH, 1),
        "b2_in": np.asarray(inputs["b2"], np.float32).reshape(1, 1),
        "bo_in": np.asarray(inputs["bo"], np.float32).reshape(1, QD),
    }
    in_maps = []
    for c in range(8):
        b, qh = c // 2, c % 2
        m = dict(common)
        m["x_in"] = np.ascontiguousarray(x[b, qh * NQ:(qh + 1) * NQ, :])
        m["ctx_in"] = np.ascontiguousarray(context[b])
        in_maps.append(m)
    return in_maps


def kernel(x, context, Wq, Wk, Wv, W1, b1, W2, b2, Wo, bo):
    x = np.ascontiguousarray(np.asarray(x, dtype=np.float32))
    context = np.ascontiguousarray(np.asarray(context, dtype=np.float32))
    B, N, _ = x.shape
    NQ = N // 2
    M = context.shape[1]
    nc = _get_compiled(NQ=NQ, M=M)
    in_maps = _make_in_maps(dict(
        x=x, context=context, Wq=Wq, Wk=Wk, Wv=Wv, W1=W1, b1=b1, W2=W2,
        b2=b2, Wo=Wo, bo=bo))

    res = run_bass_kernel_spmd(nc, in_maps, list(range(8))).results
    out = np.empty((B, N, QD), dtype=np.float32)
    for c in range(8):
        b, qh = c // 2, c % 2
        out[b, qh * NQ:(qh + 1) * NQ, :] = res[c]["y_out"]
    return out
